# revision 11
# baseline (speedup 1.0000x reference)
"""Trainium2 Bass kernel for nn_KAN_63230508532179 (dense_mlp).

Model (per reference):
  h = gelu(x[:,:,None] * bw1 + bb1)            # [B,1000,16]
  f = tanh(einsum('bnh,noh->bno', h, bw2)+bb2) # [B,1000,8]
  z = f.reshape(B, 8000)
  z = gelu(z @ wc1.T + bc1)                    # [B,256]
  z = gelu(z @ wc2.T + bc2)                    # [B,128]
  y = z @ wc3.T + bc3                          # [B,300]

Strategy: data-parallel over batch across 8 cores (512 rows each).
Each branch n is a smooth scalar map f_n: R -> R^8. We approximate it
as a degree-7 polynomial in the warped variable u = tanh(x/S0), with
per-branch coefficients C[n,o,d] from a weighted ridge least-squares
fit on a grid (host side, from the provided weights). Because the
approximation is linear in the basis u^d and combiner layer 1 is
linear, C folds into wc1 on the host:
  z1 = Wt @ U  with  Wt[m,(n,d)] = sum_o wc1[m,8n+o] C[n,o,d]
and the d=0 (constant) column folds into the bc1 bias. On device the
whole branch stack collapses to:
  - 8 ScalarE tanh activations (u tiles, [128,512])
  - a power ladder u^2..u^7 split across VectorE and GpSimd
  - 112 accumulating matmuls (K = 1024 branches x 7 powers)
  - the small combiner tail (gelu/matmul/gelu/matmul)
A few warm-up matmuls run during the input DMA window so the tensor
engine p-state is fully ramped when the real stream starts. Inputs are
repacked/padded on the host (1000 -> 1024 branches) and cast to bf16;
PSUM accumulates fp32.
"""

import os
import sys
from contextlib import ExitStack

sys.path.insert(0, "/opt/trn_rl_repo")
os.environ.setdefault("MYCRO_LOCAL_CACHE", "1")

import numpy as np
import ml_dtypes

import concourse.bass as bass
import concourse.tile as tile
from concourse import bacc, mybir
from concourse.bass_utils import run_bass_kernel_spmd

BF16 = mybir.dt.bfloat16
F32 = mybir.dt.float32
NPBF16 = ml_dtypes.bfloat16

B, N, H1, H2 = 4096, 1000, 16, 8
C1, C2, OUT = 256, 128, 300
NCORES = 8
BC = B // NCORES          # 512 batch rows per core
NP_ = 1024                # padded branches
NBT = 8                   # branch tiles of 128
DEG = 7                   # polynomial degree in u
ND = DEG                  # device basis funcs per tile (d = 1..7)
NWARM = 14                # tensor-engine warm-up matmuls

S0 = 2.2                  # u = tanh(x / S0)
FIT_GRID = 512
FIT_XMAX = 6.0
FIT_LAM = 1e-4

_CACHE = {}


def _build_program():
    if "nc" in _CACHE:
        return _CACHE["nc"]

    nc = bacc.Bacc("TRN2", target_bir_lowering=False, debug=False,
                   num_devices=NCORES)

    # x transposed per-tile: column block t = branches 128t..128t+128
    xt_d = nc.dram_tensor("xt", [128, NBT * BC], BF16, kind="ExternalInput")
    # folded comb1 weights: chunk (t,d) -> [128 branches, 256 outs], d=1..7
    wt_d = nc.dram_tensor("wt", [128, NBT * ND * C1], BF16,
                          kind="ExternalInput")
    # f32 consts: col0 = 1/S0, col1:3 = bc1, col3 = bc2, col4:7 = bc3
    cf_d = nc.dram_tensor("cf", [128, 7], F32, kind="ExternalInput")
    # bf16 consts: [0:256] wc2, [256:556] wc3
    cb_d = nc.dram_tensor("cb", [128, 256 + OUT], BF16, kind="ExternalInput")
    out_d = nc.dram_tensor("out", [OUT, BC], F32, kind="ExternalOutput")

    AF = mybir.ActivationFunctionType

    with ExitStack() as ctx:
        tc = ctx.enter_context(tile.TileContext(nc))
        consts = ctx.enter_context(tc.tile_pool(name="consts", bufs=1))
        u_pool = ctx.enter_context(tc.tile_pool(name="u", bufs=2))
        p_pool = ctx.enter_context(tc.tile_pool(name="p", bufs=8))
        z_pool = ctx.enter_context(tc.tile_pool(name="z", bufs=1))
        ps_z = ctx.enter_context(tc.tile_pool(name="psz", bufs=1, space="PSUM"))
        ps_t = ctx.enter_context(tc.tile_pool(name="pst", bufs=1, space="PSUM"))
        ps_o = ctx.enter_context(tc.tile_pool(name="pso", bufs=2, space="PSUM"))
        ps_w = ctx.enter_context(tc.tile_pool(name="psw", bufs=1, space="PSUM"))

        # ---- warm-up: ramp the PE p-state while input DMAs stream ----
        warm = consts.tile([128, BC], BF16, tag="warm")
        nc.gpsimd.memset(warm[:], 0)
        wact = consts.tile([128, 1], BF16, tag="wact")
        # triggers the tanh ACT table load early (off the critical path)
        nc.scalar.activation(wact[:], warm[:, 0:1], AF.Tanh)
        warm_ps = ps_w.tile([128, BC], F32, tag="warm")
        for _ in range(NWARM):
            nc.tensor.matmul(warm_ps[:], lhsT=warm[:, 0:128], rhs=warm[:],
                             start=True, stop=True)

        # ---- input DMAs, ordered so the pipeline starts early ----
        xt0 = consts.tile([128, BC], BF16, tag="xt0")
        nc.sync.dma_start(out=xt0[:], in_=xt_d[:, 0:BC])
        cf_sb = consts.tile([128, 7], F32, tag="cf")
        nc.sync.dma_start(out=cf_sb[:], in_=cf_d[:, :])
        wt_sb = []
        wtc = consts.tile([128, ND * C1], BF16, tag="wt0")
        nc.sync.dma_start(out=wtc[:], in_=wt_d[:, 0:ND * C1])
        wt_sb.append(wtc)
        cb_sb = consts.tile([128, 256 + OUT], BF16, tag="cb")
        nc.sync.dma_start(out=cb_sb[:], in_=cb_d[:, :])
        xtR = consts.tile([128, (NBT - 1) * BC], BF16, tag="xtR")
        nc.sync.dma_start(out=xtR[:], in_=xt_d[:, BC:NBT * BC])
        for t in range(1, NBT):
            wtc = consts.tile([128, ND * C1], BF16, tag=f"wt{t}")
            nc.sync.dma_start(
                out=wtc[:], in_=wt_d[:, ND * C1 * t:ND * C1 * (t + 1)])
            wt_sb.append(wtc)

        def xt_ap(t):
            if t == 0:
                return xt0[:]
            return xtR[:, (t - 1) * BC:t * BC]

        # ---- main loop: 8 branch tiles x 7 powers of u ----
        z1a_ps = ps_z.tile([128, BC], F32, tag="z1a")
        z1b_ps = ps_z.tile([128, BC], F32, tag="z1b")

        NK = NBT * ND

        def mm_pair(t, d, phi):
            k = t * ND + (d - 1)
            first, last = k == 0, k == NK - 1
            off = C1 * (d - 1)
            wtc = wt_sb[t]
            nc.tensor.matmul(z1a_ps[:], lhsT=wtc[:, off:off + 128],
                             rhs=phi[:], start=first, stop=last,
                             skip_group_check=True)
            nc.tensor.matmul(z1b_ps[:], lhsT=wtc[:, off + 128:off + 256],
                             rhs=phi[:], start=first, stop=last,
                             skip_group_check=True)

        for t in range(NBT):
            u = u_pool.tile([128, BC], BF16)
            nc.scalar.activation(u[:], xt_ap(t), AF.Tanh,
                                 bias=0.0, scale=cf_sb[:, 0:1])
            mm_pair(t, 1, u)
            u2 = p_pool.tile([128, BC], BF16)
            nc.vector.tensor_mul(u2[:], u[:], u[:])
            mm_pair(t, 2, u2)
            u3 = p_pool.tile([128, BC], BF16)
            nc.gpsimd.tensor_mul(u3[:], u2[:], u[:])
            mm_pair(t, 3, u3)
            u4 = p_pool.tile([128, BC], BF16)
            nc.vector.tensor_mul(u4[:], u2[:], u2[:])
            mm_pair(t, 4, u4)
            u5 = p_pool.tile([128, BC], BF16)
            nc.gpsimd.tensor_mul(u5[:], u4[:], u[:])
            mm_pair(t, 5, u5)
            u6 = p_pool.tile([128, BC], BF16)
            nc.vector.tensor_mul(u6[:], u4[:], u2[:])
            mm_pair(t, 6, u6)
            u7 = p_pool.tile([128, BC], BF16)
            nc.gpsimd.tensor_mul(u7[:], u4[:], u3[:])
            mm_pair(t, 7, u7)

        # ---- combiner tail ----
        z1a = z_pool.tile([128, BC], BF16, tag="z1a_sb")
        z1b = z_pool.tile([128, BC], BF16, tag="z1b_sb")
        nc.scalar.activation(z1a[:], z1a_ps[:], AF.Gelu,
                             bias=cf_sb[:, 1:2], scale=1.0)
        nc.scalar.activation(z1b[:], z1b_ps[:], AF.Gelu,
                             bias=cf_sb[:, 2:3], scale=1.0)

        z2_ps = ps_t.tile([128, BC], F32, tag="z2ps")
        nc.tensor.matmul(z2_ps[:], lhsT=cb_sb[:, 0:128], rhs=z1a[:],
                         start=True, stop=False, skip_group_check=True)
        nc.tensor.matmul(z2_ps[:], lhsT=cb_sb[:, 128:256], rhs=z1b[:],
                         start=False, stop=True, skip_group_check=True)
        z2 = z_pool.tile([128, BC], BF16, tag="z2_sb")
        nc.scalar.activation(z2[:], z2_ps[:], AF.Gelu,
                             bias=cf_sb[:, 3:4], scale=1.0)

        for i, m in ((0, 128), (1, 128), (2, 44)):
            o_ps = ps_o.tile([128, BC], F32, tag="ops")
            nc.tensor.matmul(o_ps[0:m, :],
                             lhsT=cb_sb[:, 256 + 128 * i:256 + 128 * i + m],
                             rhs=z2[:], start=True, stop=True)
            o_sb = z_pool.tile([128, BC], F32, tag=f"o{i}")
            nc.vector.tensor_scalar_add(o_sb[0:m, :], o_ps[0:m, :],
                                        cf_sb[0:m, 4 + i:5 + i])
            nc.sync.dma_start(out=out_d[128 * i:128 * i + m, :],
                              in_=o_sb[0:m, :])

    nc.compile()
    _CACHE["nc"] = nc
    return nc


def _gelu(a):
    from scipy.special import erf
    return 0.5 * a * (1 + erf(a / np.sqrt(2)))


def _fit_coeffs(bw1, bb1, bw2, bb2):
    """Weighted ridge lstsq fit of each branch map R->R^8 as a degree-7
    polynomial in u = tanh(x/S0). Returns C [8, N, 8] float64."""
    xs = np.linspace(-FIT_XMAX, FIT_XMAX, FIT_GRID)
    hg = _gelu(xs[None, :, None] * bw1[:, None, :].astype(np.float64)
               + bb1[:, None, :])                       # [N, G, 16]
    g = np.tanh(np.einsum('nsk,nok->nso', hg, bw2.astype(np.float64))
                + bb2[:, None, :])                      # [N, G, 8]
    wts = np.sqrt(np.exp(-xs ** 2 / 2) + 1e-3)
    ug = np.tanh(xs / S0)
    Phi = np.stack([ug ** d for d in range(DEG + 1)], 1)  # [G, 8]
    A = Phi * wts[:, None]
    Bm = (g * wts[None, :, None]).transpose(1, 0, 2).reshape(FIT_GRID, -1)
    AtA = A.T @ A + FIT_LAM * np.eye(DEG + 1)
    C = np.linalg.solve(AtA, A.T @ Bm)                  # [8, N*8]
    return C.reshape(DEG + 1, N, H2)


def preprocess(x, bw1, bb1, bw2, bb2, wc1, bc1, wc2, bc2, wc3, bc3):
    """Host-side: fit poly coefficients, fold into wc1/bc1, repack."""
    f32 = np.float32
    C = _fit_coeffs(bw1, bb1, bw2, bb2)                 # [8, N, 8]

    # fold: Wt[m, n, d] = sum_o wc1[m, 8n+o] * C[d, n, o], pad N->1024
    Wt = np.einsum('mno,rno->mnr', wc1.reshape(C1, N, H2).astype(np.float64),
                   C)                                   # [256, N, 8]
    # constant term (d=0) folds into the bc1 bias; round like the
    # device path (bf16 weights summed in fp32)
    Wt_b = Wt.astype(NPBF16).astype(np.float64)
    bias0 = Wt_b[:, :, 0].sum(axis=1)                   # [256]
    bc1f = (bc1.astype(np.float64) + bias0).astype(f32)

    Wtp = np.zeros((C1, NP_, ND))
    Wtp[:, :N, :] = Wt[:, :, 1:]
    # device layout: wt[k, ((t*ND + d-1)*256 + m)] = Wt[m, 128t+k, d]
    wt_sb = np.ascontiguousarray(
        Wtp.reshape(C1, NBT, 128, ND).transpose(2, 1, 3, 0).reshape(
            128, NBT * ND * C1)).astype(NPBF16)

    # x transposed, padded to 1024 rows, tile-major [128, 8*B]
    xq = np.zeros((NP_, B), f32)
    xq[:N] = x.T
    xq = xq.astype(NPBF16).reshape(NBT, 128, B)

    cf = np.zeros((128, 7), f32)
    cf[:, 0] = 1.0 / S0
    cf[:, 1:3] = bc1f.reshape(2, 128).T
    cf[:, 3] = bc2
    bc3p = np.zeros(384, f32); bc3p[:OUT] = bc3
    cf[:, 4:7] = bc3p.reshape(3, 128).T

    cb = np.empty((128, 256 + OUT), NPBF16)
    cb[:, 0:256] = wc2.T.reshape(2, 128, C2).transpose(1, 0, 2).reshape(
        128, 256).astype(NPBF16)
    cb[:, 256:] = wc3.T.astype(NPBF16)

    shared = {"wt": wt_sb, "cf": cf, "cb": np.ascontiguousarray(cb)}
    in_maps = []
    for c in range(NCORES):
        m = dict(shared)
        m["xt"] = np.ascontiguousarray(
            xq[:, :, BC * c:BC * (c + 1)].transpose(1, 0, 2).reshape(
                128, NBT * BC))
        in_maps.append(m)
    return in_maps


def run(in_maps, trace=False):
    nc = _build_program()
    return run_bass_kernel_spmd(nc, in_maps, list(range(NCORES)), trace=trace)


def kernel(x, bw1, bb1, bw2, bb2, wc1, bc1, wc2, bc2, wc3, bc3):
    args = [np.asarray(a, np.float32) for a in
            (x, bw1, bb1, bw2, bb2, wc1, bc1, wc2, bc2, wc3, bc3)]
    in_maps = preprocess(*args)
    res = run(in_maps, trace=False)
    y = np.empty((B, OUT), np.float32)
    for c in range(NCORES):
        y[BC * c:BC * (c + 1), :] = res.results[c]["out"].T
    return y


# revision 13
# speedup vs baseline: 1.1331x; 1.1331x over previous
"""Trainium2 Bass kernel for nn_KAN_63230508532179 (dense_mlp).

Model (per reference):
  h = gelu(x[:,:,None] * bw1 + bb1)            # [B,1000,16]
  f = tanh(einsum('bnh,noh->bno', h, bw2)+bb2) # [B,1000,8]
  z = f.reshape(B, 8000)
  z = gelu(z @ wc1.T + bc1)                    # [B,256]
  z = gelu(z @ wc2.T + bc2)                    # [B,128]
  y = z @ wc3.T + bc3                          # [B,300]

Strategy: data-parallel over batch across 8 cores (512 rows each).
Each branch n is a smooth scalar map f_n: R -> R^8. We approximate it
as a degree-7 polynomial in the warped variable u = tanh(x/S0), with
per-branch coefficients C[n,o,d] from a weighted ridge least-squares
fit on a grid (host side, from the provided weights). Because the
approximation is linear in the basis u^d and combiner layer 1 is
linear, C folds into wc1 on the host:
  z1 = Wt @ U  with  Wt[m,(n,d)] = sum_o wc1[m,8n+o] C[n,o,d]
and the d=0 (constant) column folds into the bc1 bias. On device the
whole branch stack collapses to:
  - 8 ScalarE tanh activations (u tiles, [128,512])
  - a power ladder u^2..u^7 split across VectorE and GpSimd
  - 112 accumulating matmuls (K = 1024 branches x 7 powers)
  - the small combiner tail (gelu/matmul/gelu/matmul)
A few warm-up matmuls run during the input DMA window so the tensor
engine p-state is fully ramped when the real stream starts. Inputs are
repacked/padded on the host (1000 -> 1024 branches) and cast to bf16;
PSUM accumulates fp32.
"""

import os
import sys
from contextlib import ExitStack

sys.path.insert(0, "/opt/trn_rl_repo")
os.environ.setdefault("MYCRO_LOCAL_CACHE", "1")

import numpy as np
import ml_dtypes

import concourse.bass as bass
import concourse.tile as tile
from concourse import bacc, mybir
from concourse.bass_utils import run_bass_kernel_spmd

BF16 = mybir.dt.bfloat16
F32 = mybir.dt.float32
NPBF16 = ml_dtypes.bfloat16

B, N, H1, H2 = 4096, 1000, 16, 8
C1, C2, OUT = 256, 128, 300
NCORES = 8
BC = B // NCORES          # 512 batch rows per core
NP_ = 1024                # padded branches
NBT = 8                   # branch tiles of 128
DEG = 7                   # polynomial degree in u
ND = DEG                  # device basis funcs per tile (d = 1..7)
NWARM = 14                # tensor-engine warm-up matmuls

S0 = 2.2                  # u = tanh(x / S0)
FIT_GRID = 512
FIT_XMAX = 6.0
FIT_LAM = 1e-4

_CACHE = {}


def _build_program():
    if "nc" in _CACHE:
        return _CACHE["nc"]

    nc = bacc.Bacc("TRN2", target_bir_lowering=False, debug=False,
                   num_devices=NCORES)

    # x transposed per-tile: column block t = branches 128t..128t+128
    xt_d = nc.dram_tensor("xt", [128, NBT * BC], BF16, kind="ExternalInput")
    # folded comb1 weights: chunk (t,d) -> [128 branches, 256 outs], d=1..7
    wt_d = nc.dram_tensor("wt", [128, NBT * ND * C1], BF16,
                          kind="ExternalInput")
    # f32 consts: col0 = 1/S0, col1:3 = bc1, col3 = bc2, col4:7 = bc3
    cf_d = nc.dram_tensor("cf", [128, 7], F32, kind="ExternalInput")
    # bf16 consts: [0:256] wc2, [256:556] wc3
    cb_d = nc.dram_tensor("cb", [128, 256 + OUT], BF16, kind="ExternalInput")
    out_d = nc.dram_tensor("out", [OUT, BC], F32, kind="ExternalOutput")

    AF = mybir.ActivationFunctionType

    with ExitStack() as ctx:
        tc = ctx.enter_context(tile.TileContext(nc))
        consts = ctx.enter_context(tc.tile_pool(name="consts", bufs=1))
        p_pool = ctx.enter_context(tc.tile_pool(name="p", bufs=2))
        z_pool = ctx.enter_context(tc.tile_pool(name="z", bufs=1))
        ps_z = ctx.enter_context(tc.tile_pool(name="psz", bufs=1, space="PSUM"))
        ps_t = ctx.enter_context(tc.tile_pool(name="pst", bufs=1, space="PSUM"))
        ps_o = ctx.enter_context(tc.tile_pool(name="pso", bufs=2, space="PSUM"))
        ps_w = ctx.enter_context(tc.tile_pool(name="psw", bufs=1, space="PSUM"))

        # ---- warm-up: ramp the PE p-state while input DMAs stream ----
        warm = consts.tile([128, BC], BF16, tag="warm")
        nc.gpsimd.memset(warm[:], 0)
        wact = consts.tile([128, 1], BF16, tag="wact")
        # triggers the tanh ACT table load early (off the critical path)
        nc.scalar.activation(wact[:], warm[:, 0:1], AF.Tanh)
        warm_ps = ps_w.tile([128, BC], F32, tag="warm")
        for _ in range(NWARM):
            nc.tensor.matmul(warm_ps[:], lhsT=warm[:, 0:128], rhs=warm[:],
                             start=True, stop=True)

        # ---- input DMAs, ordered so the pipeline starts early ----
        xt0 = consts.tile([128, BC], BF16, tag="xt0")
        nc.sync.dma_start(out=xt0[:], in_=xt_d[:, 0:BC])
        cf_sb = consts.tile([128, 7], F32, tag="cf")
        nc.sync.dma_start(out=cf_sb[:], in_=cf_d[:, :])
        wt_sb = []
        wtc = consts.tile([128, ND * C1], BF16, tag="wt0")
        nc.sync.dma_start(out=wtc[:], in_=wt_d[:, 0:ND * C1])
        wt_sb.append(wtc)
        cb_sb = consts.tile([128, 256 + OUT], BF16, tag="cb")
        nc.sync.dma_start(out=cb_sb[:], in_=cb_d[:, :])
        xtR = consts.tile([128, (NBT - 1) * BC], BF16, tag="xtR")
        nc.sync.dma_start(out=xtR[:], in_=xt_d[:, BC:NBT * BC])
        for t in range(1, NBT):
            wtc = consts.tile([128, ND * C1], BF16, tag=f"wt{t}")
            nc.sync.dma_start(
                out=wtc[:], in_=wt_d[:, ND * C1 * t:ND * C1 * (t + 1)])
            wt_sb.append(wtc)

        def xt_ap(t):
            if t == 0:
                return xt0[:]
            return xtR[:, (t - 1) * BC:t * BC]

        # ---- main loop: 8 branch tiles x 7 powers of u ----
        z1a_ps = ps_z.tile([128, BC], F32, tag="z1a")
        z1b_ps = ps_z.tile([128, BC], F32, tag="z1b")

        NK = NBT * ND

        def mm_pair(t, d, phi):
            k = t * ND + (d - 1)
            first, last = k == 0, k == NK - 1
            off = C1 * (d - 1)
            wtc = wt_sb[t]
            nc.tensor.matmul(z1a_ps[:], lhsT=wtc[:, off:off + 128],
                             rhs=phi[:], start=first, stop=last,
                             skip_group_check=True)
            nc.tensor.matmul(z1b_ps[:], lhsT=wtc[:, off + 128:off + 256],
                             rhs=phi[:], start=first, stop=last,
                             skip_group_check=True)

        PACKED = True
        for t in range(NBT):
            # pw blocks: 0:u 1:u2 2:u3 3:u4 4:u5 5:u6 6:u7
            pw = p_pool.tile([128, ND * BC], BF16)
            blk = lambda a, b: pw[:, a * BC:b * BC]
            nc.scalar.activation(blk(0, 1), xt_ap(t), AF.Tanh,
                                 bias=0.0, scale=cf_sb[:, 0:1])
            mm_pair(t, 1, blk(0, 1))
            nc.vector.tensor_mul(blk(1, 2), blk(0, 1), blk(0, 1))
            mm_pair(t, 2, blk(1, 2))
            if PACKED:
                u2rep = blk(1, 2).unsqueeze(1).broadcast_to([128, 2, BC])
                nc.vector.tensor_mul(
                    blk(2, 4).rearrange("p (a b) -> p a b", a=2),
                    blk(0, 2).rearrange("p (a b) -> p a b", a=2), u2rep)
                mm_pair(t, 3, blk(2, 3))
                mm_pair(t, 4, blk(3, 4))
                nc.vector.tensor_mul(
                    blk(4, 6).rearrange("p (a b) -> p a b", a=2),
                    blk(2, 4).rearrange("p (a b) -> p a b", a=2), u2rep)
                mm_pair(t, 5, blk(4, 5))
                mm_pair(t, 6, blk(5, 6))
            else:
                nc.vector.tensor_mul(blk(2, 3), blk(1, 2), blk(0, 1))
                mm_pair(t, 3, blk(2, 3))
                nc.vector.tensor_mul(blk(3, 4), blk(1, 2), blk(1, 2))
                mm_pair(t, 4, blk(3, 4))
                nc.vector.tensor_mul(blk(4, 5), blk(3, 4), blk(0, 1))
                mm_pair(t, 5, blk(4, 5))
                nc.vector.tensor_mul(blk(5, 6), blk(3, 4), blk(1, 2))
                mm_pair(t, 6, blk(5, 6))
            nc.vector.tensor_mul(blk(6, 7), blk(4, 5), blk(1, 2))
            mm_pair(t, 7, blk(6, 7))

        # ---- combiner tail ----
        z1a = z_pool.tile([128, BC], BF16, tag="z1a_sb")
        z1b = z_pool.tile([128, BC], BF16, tag="z1b_sb")
        nc.scalar.activation(z1a[:], z1a_ps[:], AF.Gelu,
                             bias=cf_sb[:, 1:2], scale=1.0)
        nc.scalar.activation(z1b[:], z1b_ps[:], AF.Gelu,
                             bias=cf_sb[:, 2:3], scale=1.0)

        z2_ps = ps_t.tile([128, BC], F32, tag="z2ps")
        nc.tensor.matmul(z2_ps[:], lhsT=cb_sb[:, 0:128], rhs=z1a[:],
                         start=True, stop=False, skip_group_check=True)
        nc.tensor.matmul(z2_ps[:], lhsT=cb_sb[:, 128:256], rhs=z1b[:],
                         start=False, stop=True, skip_group_check=True)
        z2 = z_pool.tile([128, BC], BF16, tag="z2_sb")
        nc.scalar.activation(z2[:], z2_ps[:], AF.Gelu,
                             bias=cf_sb[:, 3:4], scale=1.0)

        for i, m in ((0, 128), (1, 128), (2, 44)):
            o_ps = ps_o.tile([128, BC], F32, tag="ops")
            nc.tensor.matmul(o_ps[0:m, :],
                             lhsT=cb_sb[:, 256 + 128 * i:256 + 128 * i + m],
                             rhs=z2[:], start=True, stop=True)
            o_sb = z_pool.tile([128, BC], F32, tag=f"o{i}")
            nc.vector.tensor_scalar_add(o_sb[0:m, :], o_ps[0:m, :],
                                        cf_sb[0:m, 4 + i:5 + i])
            nc.sync.dma_start(out=out_d[128 * i:128 * i + m, :],
                              in_=o_sb[0:m, :])

    nc.compile()
    _CACHE["nc"] = nc
    return nc


def _gelu(a):
    from scipy.special import erf
    return 0.5 * a * (1 + erf(a / np.sqrt(2)))


def _fit_coeffs(bw1, bb1, bw2, bb2):
    """Weighted ridge lstsq fit of each branch map R->R^8 as a degree-7
    polynomial in u = tanh(x/S0). Returns C [8, N, 8] float64."""
    xs = np.linspace(-FIT_XMAX, FIT_XMAX, FIT_GRID)
    hg = _gelu(xs[None, :, None] * bw1[:, None, :].astype(np.float64)
               + bb1[:, None, :])                       # [N, G, 16]
    g = np.tanh(np.einsum('nsk,nok->nso', hg, bw2.astype(np.float64))
                + bb2[:, None, :])                      # [N, G, 8]
    wts = np.sqrt(np.exp(-xs ** 2 / 2) + 1e-3)
    ug = np.tanh(xs / S0)
    Phi = np.stack([ug ** d for d in range(DEG + 1)], 1)  # [G, 8]
    A = Phi * wts[:, None]
    Bm = (g * wts[None, :, None]).transpose(1, 0, 2).reshape(FIT_GRID, -1)
    AtA = A.T @ A + FIT_LAM * np.eye(DEG + 1)
    C = np.linalg.solve(AtA, A.T @ Bm)                  # [8, N*8]
    return C.reshape(DEG + 1, N, H2)


def preprocess(x, bw1, bb1, bw2, bb2, wc1, bc1, wc2, bc2, wc3, bc3):
    """Host-side: fit poly coefficients, fold into wc1/bc1, repack."""
    f32 = np.float32
    C = _fit_coeffs(bw1, bb1, bw2, bb2)                 # [8, N, 8]

    # fold: Wt[m, n, d] = sum_o wc1[m, 8n+o] * C[d, n, o], pad N->1024
    Wt = np.einsum('mno,rno->mnr', wc1.reshape(C1, N, H2).astype(np.float64),
                   C)                                   # [256, N, 8]
    # constant term (d=0) folds into the bc1 bias; round like the
    # device path (bf16 weights summed in fp32)
    Wt_b = Wt.astype(NPBF16).astype(np.float64)
    bias0 = Wt_b[:, :, 0].sum(axis=1)                   # [256]
    bc1f = (bc1.astype(np.float64) + bias0).astype(f32)

    Wtp = np.zeros((C1, NP_, ND))
    Wtp[:, :N, :] = Wt[:, :, 1:]
    # device layout: wt[k, ((t*ND + d-1)*256 + m)] = Wt[m, 128t+k, d]
    wt_sb = np.ascontiguousarray(
        Wtp.reshape(C1, NBT, 128, ND).transpose(2, 1, 3, 0).reshape(
            128, NBT * ND * C1)).astype(NPBF16)

    # x transposed, padded to 1024 rows, tile-major [128, 8*B]
    xq = np.zeros((NP_, B), f32)
    xq[:N] = x.T
    xq = xq.astype(NPBF16).reshape(NBT, 128, B)

    cf = np.zeros((128, 7), f32)
    cf[:, 0] = 1.0 / S0
    cf[:, 1:3] = bc1f.reshape(2, 128).T
    cf[:, 3] = bc2
    bc3p = np.zeros(384, f32); bc3p[:OUT] = bc3
    cf[:, 4:7] = bc3p.reshape(3, 128).T

    cb = np.empty((128, 256 + OUT), NPBF16)
    cb[:, 0:256] = wc2.T.reshape(2, 128, C2).transpose(1, 0, 2).reshape(
        128, 256).astype(NPBF16)
    cb[:, 256:] = wc3.T.astype(NPBF16)

    shared = {"wt": wt_sb, "cf": cf, "cb": np.ascontiguousarray(cb)}
    in_maps = []
    for c in range(NCORES):
        m = dict(shared)
        m["xt"] = np.ascontiguousarray(
            xq[:, :, BC * c:BC * (c + 1)].transpose(1, 0, 2).reshape(
                128, NBT * BC))
        in_maps.append(m)
    return in_maps


def run(in_maps, trace=False):
    nc = _build_program()
    return run_bass_kernel_spmd(nc, in_maps, list(range(NCORES)), trace=trace)


def kernel(x, bw1, bb1, bw2, bb2, wc1, bc1, wc2, bc2, wc3, bc3):
    args = [np.asarray(a, np.float32) for a in
            (x, bw1, bb1, bw2, bb2, wc1, bc1, wc2, bc2, wc3, bc3)]
    in_maps = preprocess(*args)
    res = run(in_maps, trace=False)
    y = np.empty((B, OUT), np.float32)
    for c in range(NCORES):
        y[BC * c:BC * (c + 1), :] = res.results[c]["out"].T
    return y
